# revision 35
# baseline (speedup 1.0000x reference)
"""Trainium2 Bass kernel for nn_BetweenClusterFC.

Computes out[e] = (emb_1[f[e]] @ W1 + b1) . (emb_2[t[e]] @ W2 + b2)
for E = 1.6M edges over N = 100k nodes, D_IN = 256, D_OUT = 128.

Strategy (8 NeuronCores, SPMD, full inputs in / full output out):
  - Core (a, b) (a = f//50000, b = t//25000) handles edges with from-node in
    half a and to-node in quarter b.  Within a core, from-nodes form 4 blocks
    of 12500 (padded 12800) and to-nodes 4 blocks of 6250 (padded 6656);
    edges are bucketed by (from-block, to-block) -> 16 buckets.
  - PE projects both tables in bf16 (bias folded as btile-add on early
    drains / rank-1 matmul on late groups); drains go to bf16 and are
    written to DRAM tables typed uint32 (2 bf16 packed per element), which
    halves the modeled Pool cost of the row gathers (cost tracks the
    gather's out free-size, and the SWDGE ring caps each call at 1024 rows).
  - Per bucket, edges are fetched with 1024-row SWDGE dma_gather calls into
    4096-edge call buffers; dot products run as a deep software pipeline:
    DVE does mul + first three add-tree levels (bf16, 2x mode), the Pool
    engine (free mlp<->standard gpsimd library switches) folds the last
    add level and the reduce-over-8, one op per call per iteration so no
    consecutive ops share a dependency chain.
  - The host applies the inverse edge permutation to assemble the output.

Per-engine budget (core, modeled): Pool ~= gathers 168us + L3/L4 32us; DVE
~= mul 109us + L1/L2/red8 103us + early drains; Act ~= et halves + late
drains + idx ~130us; SP ~= et halves + p-writes + res ~138us; PE ~= 120us.
Modeled total 275.4us (baseline 381.3us).
"""

import contextlib

import numpy as np
import ml_dtypes

import concourse.bass as bass
import concourse.mybir as mybir

# ---------------------------------------------------------------- constants
N_NODES = 100_000
D_IN = 256
D_OUT = 128
N_EDGES = 1_600_000
N_CORES = 8

NFB = 4              # from-blocks per core
NTB = 4              # to-blocks per core
NB1 = 12_500         # from nodes per block
NBP1 = 12_800        # padded (25 * 512)
NB2 = 6_250          # to nodes per block
NBP2 = 6_656         # padded (13 * 512)
NBUCKET = NFB * NTB  # 16

P1_ROWS = NFB * NBP1   # 51200
P2_ROWS = NTB * NBP2   # 26624
G1_PER_B = NBP1 // 512   # 25 groups per from-block
G2_PER_B = NBP2 // 512   # 13 groups per to-block
GROUPS1 = NFB * G1_PER_B  # 100
GROUPS2 = NTB * G2_PER_B  # 52
NGROUP = GROUPS1 + GROUPS2  # 152

TILES1 = P1_ROWS // 128   # 400
TILES2 = P2_ROWS // 128   # 208
CHUNK_T1 = 10             # tiles per chunk, table 1 (100 % 10 == 0)
CHUNK_T2 = 13             # tiles per chunk, table 2 (52 % 13 == 0)
NCH1 = TILES1 // CHUNK_T1  # 40
NCH2 = TILES2 // CHUNK_T2  # 16
NCHUNK = NCH1 + NCH2       # 56
ECOLS_MAX = 1664
ECOLS1 = CHUNK_T1 * 128    # 1280
ECOLS2 = CHUNK_T2 * 128    # 1664

CALL = 4096          # edges per dot-product call
GS = 1024            # rows per dma_gather (SWDGE ring capacity)
DU = 64              # u32 elems per 128-bf16 row
AT_BUFS = 9
BT_BUFS = 3
DSPLIT = 54          # groups below drain on DVE (idle until first gathers)
PVS = 24             # pv ring slots (512 bf16 cols each)

F32 = mybir.dt.float32
BF16 = mybir.dt.bfloat16
I16 = mybir.dt.int16
U32 = mybir.dt.uint32
AX = mybir.AxisListType
BFNP = ml_dtypes.bfloat16

# Group processing order: alternate t-block / f-block so bucket (0,0) is
# ready after 38 groups.  Global group ids: table1 block fi = [25fi, 25fi+25);
# table2 block ti = [100+13ti, ...).
GSEQ = []
_RUNS = []  # (start_pos, length) of same-block runs
for _i in range(4):
    _RUNS.append((len(GSEQ), G2_PER_B))
    GSEQ += list(range(GROUPS1 + G2_PER_B * _i, GROUPS1 + G2_PER_B * (_i + 1)))
    _RUNS.append((len(GSEQ), G1_PER_B))
    GSEQ += list(range(G1_PER_B * _i, G1_PER_B * (_i + 1)))
RUN_OF = {}
for _s, _l in _RUNS:
    for _q in range(_s, _s + _l):
        RUN_OF[_q] = _s

# position in GSEQ after which block is fully projected
_f_done = {}
_t_done = {}
for _q, _g in enumerate(GSEQ):
    if _g < GROUPS1:
        _f_done[_g // G1_PER_B] = _q
    else:
        _t_done[(_g - GROUPS1) // G2_PER_B] = _q
# bucket bk=(fi*NTB+ti) ready after this many processed groups
READY_Q = [max(_f_done[bk // NTB], _t_done[bk % NTB]) + 1 for bk in range(NBUCKET)]
# bucket processing order: by readiness, then bucket id
BORDER = sorted(range(NBUCKET), key=lambda bk: (READY_Q[bk], bk))
CRANK = {bk: r for r, bk in enumerate(BORDER)}

# ---- chunk bookkeeping (per-table chunks, never spanning blocks) ----------
def _chunk_of_tile(tg):
    """global tile id -> (global chunk id, col0 within chunk)."""
    if tg < TILES1:
        return tg // CHUNK_T1, (tg % CHUNK_T1) * 128
    t2 = tg - TILES1
    return NCH1 + t2 // CHUNK_T2, (t2 % CHUNK_T2) * 128


def _chunk_src(c):
    """global chunk id -> (table, col0, ncols)."""
    if c < NCH1:
        return 0, c * ECOLS1, ECOLS1
    return 1, (c - NCH1) * ECOLS2, ECOLS2


# first/last GSEQ position touching each chunk
_first_pos = {}
_last_pos = {}
for _q, _g in enumerate(GSEQ):
    for _j in range(4):
        _tg = _g * 4 + _j if _g < GROUPS1 else TILES1 + (_g - GROUPS1) * 4 + _j
        _c, _ = _chunk_of_tile(_tg)
        _first_pos.setdefault(_c, _q)
        _last_pos[_c] = _q
CSEQ = sorted(range(NCHUNK), key=lambda c: _first_pos[c])
CPOS = {c: i for i, c in enumerate(CSEQ)}

# ---- drain plans ----------------------------------------------------------
def _mk_plan(lo, hi):
    """Pair (q, q+1) when same block-run, q even (psum-tensor adjacency)."""
    plan = []
    q = lo
    while q < hi:
        n = 2 if (q + 1 < hi and q % 2 == 0 and RUN_OF[q] == RUN_OF[q + 1]) else 1
        plan.append((q, n))
        q += n
    dmap = {}
    for i, (q0, n) in enumerate(plan):
        for p in range(q0, q0 + n):
            dmap[p] = i + 1
    return plan, dmap


DD_PLAN, DD_MAP = _mk_plan(0, DSPLIT)
DA_PLAN, DA_MAP = _mk_plan(DSPLIT, NGROUP)

# ---- p-write plan (pairs of adjacent groups, same run, pv slots adjacent) -
PW_DMAS = []
PW_JMAP = {}
_q = 0
while _q < NGROUP:
    pair = (_q + 1 < NGROUP and RUN_OF[_q] == RUN_OF[_q + 1]
            and (_q % PVS) + 1 < PVS)
    PW_DMAS.append((_q, 2 if pair else 1))
    PW_JMAP[_q] = len(PW_DMAS) - 1
    if pair:
        PW_JMAP[_q + 1] = len(PW_DMAS) - 1
    _q += 2 if pair else 1
# pw dma j -> (sem rotation, count on that sem up to and including j)
PW_COUNT = {}
for _p, _j in PW_JMAP.items():
    PW_COUNT[_p] = (_j % 8, sum(1 for j2 in range(_j + 1) if j2 % 8 == _j % 8))

# per-bucket pw gate: counts per sem rotation covering all pw dmas with
# index <= max pw index over the bucket's two blocks' groups
BUCKET_PW_NEED = []
for bk in range(NBUCKET):
    fi, ti = bk // NTB, bk % NTB
    qs = [q for q, g in enumerate(GSEQ)
          if (g < GROUPS1 and g // G1_PER_B == fi)
          or (g >= GROUPS1 and (g - GROUPS1) // G2_PER_B == ti)]
    jmax = max(PW_JMAP[q] for q in qs)
    need = [0] * 8
    for j in range(jmax + 1):
        need[j % 8] += 1
    BUCKET_PW_NEED.append(need)


def _make_calls(cap):
    n = -(-cap // CALL)
    base = cap // n // 128 * 128
    calls = [base] * n
    rem = cap - base * n
    for i in range(rem // 128):
        calls[i] += 128
    assert sum(calls) == cap and all(c <= CALL for c in calls)
    return calls


def _layout(caps):
    """Per-bucket call/slot/col bookkeeping shared by device + host code.

    idx columns are laid out in BORDER (processing) order so the two idx
    DMA parts cover prefixes of the processing sequence."""
    bcalls = [_make_calls(c) for c in caps]
    bslot_tot = [c // 128 for c in caps]
    col_off_rank = np.concatenate(
        [[0], np.cumsum([caps[bk] // 16 for bk in BORDER])]).astype(int)
    col0 = {bk: int(col_off_rank[CRANK[bk]]) for bk in range(NBUCKET)}
    slot_max = max(bslot_tot)
    # flat call list in bucket processing order
    flat = []
    for bk in BORDER:
        g0 = 0
        for ci, gsz in enumerate(bcalls[bk]):
            flat.append((bk, ci, gsz, g0))
            g0 += gsz
    red_done = {}
    cnt = 0
    for bk, ci, gsz, g0 in flat:
        cnt += 1
        red_done[bk] = cnt
    # cumulative at/bt gather-sem targets per call (16 per sub-gather)
    nsub = [-(-gsz // GS) for _, _, gsz, _ in flat]
    ga_tgt, gb_tgt = [], []
    at_cnt = [0] * AT_BUFS
    bt_cnt = [0] * BT_BUFS
    for k in range(len(flat)):
        at_cnt[k % AT_BUFS] += nsub[k]
        bt_cnt[k % BT_BUFS] += nsub[k]
        ga_tgt.append(16 * at_cnt[k % AT_BUFS])
        gb_tgt.append(16 * bt_cnt[k % BT_BUFS])
    # L2 engine assignment: some calls' L2 runs on Pool (balance)
    pl2 = [False for k in range(len(flat))]
    l2cnt = []  # cumulative same-engine L2 count through call k
    nd = npo = 0
    for k in range(len(flat)):
        if pl2[k]:
            npo += 1
            l2cnt.append(npo)
        else:
            nd += 1
            l2cnt.append(nd)
    return (bcalls, bslot_tot, col_off_rank, col0, slot_max, flat, red_done,
            nsub, ga_tgt, gb_tgt, pl2, l2cnt)


# ---------------------------------------------------------------- device code
def build_bass(caps):
    (bcalls, bslot_tot, col_off_rank, col0_of, slot_max, flat, red_done,
     nsub, ga_tgt, gb_tgt, pl2, l2cnt) = _layout(caps)
    idx_cols = int(col_off_rank[-1])
    ncall = len(flat)

    nc = bass.Bass()

    e1t = nc.dram_tensor("e1t", [D_IN, P1_ROWS], BF16, kind="ExternalInput")
    e2t = nc.dram_tensor("e2t", [D_IN, P2_ROWS], BF16, kind="ExternalInput")
    w1 = nc.dram_tensor("w1", [D_IN, D_OUT], BF16, kind="ExternalInput")
    w2 = nc.dram_tensor("w2", [D_IN, D_OUT], BF16, kind="ExternalInput")
    b1f = nc.dram_tensor("b1f", [1, 512], BF16, kind="ExternalInput")
    b2f = nc.dram_tensor("b2f", [1, 512], BF16, kind="ExternalInput")
    b1t = nc.dram_tensor("b1t", [128, 1024], BF16, kind="ExternalInput")
    b2t = nc.dram_tensor("b2t", [128, 1024], BF16, kind="ExternalInput")
    onesd = nc.dram_tensor("onesd", [1, 128], BF16, kind="ExternalInput")
    idxa = nc.dram_tensor("idxa", [128, idx_cols], I16, kind="ExternalInput")
    idxb = nc.dram_tensor("idxb", [128, idx_cols], I16, kind="ExternalInput")
    res = nc.dram_tensor("res", [NBUCKET, 128, slot_max], BF16,
                         kind="ExternalOutput")

    p1d = nc.dram_tensor("p1d", [P1_ROWS, DU], U32, kind="Internal")
    p2d = nc.dram_tensor("p2d", [P2_ROWS, DU], U32, kind="Internal")
    pdst = (p1d, p2d)

    st = contextlib.ExitStack()
    with st:
        sb = lambda nm, shape, dt=BF16: st.enter_context(nc.sbuf_tensor(nm, shape, dt))
        sem = lambda nm: st.enter_context(nc.semaphore(name=nm))

        w1c = sb("w1c", [128, 256])
        w2c = sb("w2c", [128, 256])
        bt = (sb("bt1", [1, 512]), sb("bt2", [1, 512]))
        btile = (sb("btile1", [128, 1024]), sb("btile2", [128, 1024]))
        onesr = sb("onesr", [1, 128])
        idxt = (sb("idxta", [128, idx_cols], I16), sb("idxtb", [128, idx_cols], I16))
        et = [[sb(f"et_{p}_{k}", [128, ECOLS_MAX]) for k in range(2)]
              for p in range(4)]  # [CSEQ-pos mod 4][din-half]
        pvall = sb("pvall", [128, PVS * 512])
        pv = [pvall[:, i * 512:(i + 1) * 512] for i in range(PVS)]
        ps = [st.enter_context(nc.psum_tensor(f"ps{i}", [128, 1024], F32))
              for i in range(4)]

        def psv(q, lo=0, hi=512):
            h = (q % 2) * 512
            return ps[(q % 8) // 2][:, h + lo:h + hi]

        at = [sb(f"at{i}", [128, CALL // 2], U32) for i in range(AT_BUFS)]
        btg = [sb(f"btg{i}", [128, CALL // 2], U32) for i in range(BT_BUFS)]
        rt = [sb(f"rt{bk}", [128, bslot_tot[bk]]) for bk in range(NBUCKET)]

        s_cl = sem("s_cl")               # bt/onesr consts (3 dmas -> 48)
        s_cw = sem("s_cw")               # W tiles (4 dmas -> 64)
        s_cb = sem("s_cb")               # btile tiles (2 dmas -> 32)
        s_idx1 = sem("s_idx1")           # idx ranks 0-1 (2 dmas -> 32)
        s_idx2 = sem("s_idx2")           # idx ranks 2-7 (2 dmas -> 32)
        s_idx3 = sem("s_idx3")           # idx ranks 8-15 (2 dmas -> 32)
        s_load = tuple(sem(f"s_load{i}") for i in range(4))  # chunks, by pos%4
        s_mm = sem("s_mm")               # K-matmuls (+1; 8 per group)
        s_bias = sem("s_bias")           # bias matmuls (+1; 4 per late group)
        s_dd = sem("s_dd")               # DVE drains (+1 per plan entry)
        s_da = sem("s_da")               # Act drains (+1 per plan entry)
        s_pw = tuple(sem(f"s_pw{i}") for i in range(8))   # p-writes (+16)
        s_ga = tuple(sem(f"s_ga{i}") for i in range(AT_BUFS))  # at gathers
        s_gb = tuple(sem(f"s_gb{i}") for i in range(BT_BUFS))  # bt gathers
        s_mul = sem("s_mul")             # DVE muls (+1)
        s_t0 = sem("s_t0")               # L1 (+1, DVE)
        s_t1d = sem("s_t1d")             # L2 on DVE (+1)
        s_t1p = sem("s_t1p")             # L2 on Pool (+1)
        s_t2 = sem("s_t2")               # L3 (+1, Pool)
        s_t3 = sem("s_t3")               # L4 (+1, Pool)
        s_red = sem("s_red")             # reduces (+1)
        s_out = sem("s_out")             # res dmas (+16)

        block = st.enter_context(nc.Block())

        def wait_write(eng, p):
            """Wait until group p's p-write has completed (pv slot free)."""
            r, n = PW_COUNT[p]
            eng.wait_ge(s_pw[r], 16 * n)

        def wait_drained(eng, q):
            """Wait until group position q's PSUM->pv drain has completed."""
            if q < DSPLIT:
                eng.wait_ge(s_dd, DD_MAP[q])
            else:
                eng.wait_ge(s_da, DA_MAP[q])

        def make_load_chunk(eng, half):
            def load_chunk(cq):
                cid = CSEQ[cq]
                if cq >= 4:
                    # slot cq%4 currently holds CSEQ[cq-4]; wait consumed
                    eng.wait_ge(s_mm, 8 * (_last_pos[CSEQ[cq - 4]] + 1))
                tab, col0, ncols = _chunk_src(cid)
                src = e1t if tab == 0 else e2t
                eng.dma_start(
                    out=et[cq % 4][half][:, 0:ncols],
                    in_=src[half * 128:(half + 1) * 128, col0:col0 + ncols],
                ).then_inc(s_load[cq % 4], 16)
            return load_chunk

        def run_load_loop(eng, half, body, loads_first=False):
            """Per-group body + lookahead chunk loads (chunks 0/1 are
            loaded by the engine preamble before consts)."""
            load_chunk = make_load_chunk(eng, half)
            next_cq = 2
            for q in range(NGROUP):
                if loads_first:
                    while next_cq < NCHUNK and _first_pos[CSEQ[next_cq]] <= q + 8:
                        load_chunk(next_cq)
                        next_cq += 1
                    body(q)
                else:
                    body(q)
                    while next_cq < NCHUNK and _first_pos[CSEQ[next_cq]] <= q + 8:
                        load_chunk(next_cq)
                        next_cq += 1

        # ------------------------------------------------ SP: consts, embT
        # half-0, idxa, p-writes, res stores
        @block.sync
        def _(sync):
            lc0 = make_load_chunk(sync, 0)
            lc0(0)
            lc0(1)
            for k in range(2):
                sync.dma_start(out=w1c[:, k * 128:(k + 1) * 128],
                               in_=w1[k * 128:(k + 1) * 128, :]).then_inc(s_cw, 16)
            sync.dma_start(out=btile[0][:], in_=b1t[:]).then_inc(s_cb, 16)
            sync.dma_start(out=bt[0][:], in_=b1f[:]).then_inc(s_cl, 16)
            c4 = int(col_off_rank[min(4, NBUCKET)])
            sync.dma_start(out=idxt[0][:, 0:c4],
                           in_=idxa[:, 0:c4]).then_inc(s_idx1, 16)

            pw_seen = [0] * 8
            pw_next = [0]  # index into PW_DMAS

            def body(q):
                # emit p-write j once its (drain-order) groups <= q
                while pw_next[0] < len(PW_DMAS):
                    q0, n = PW_DMAS[pw_next[0]]
                    if q0 + n - 1 > q:
                        break
                    j = pw_next[0]
                    pw_next[0] += 1
                    for p in range(q0, q0 + n):
                        wait_drained(sync, p)
                    if pw_seen[j % 8]:
                        sync.wait_ge(s_pw[j % 8], 16 * pw_seen[j % 8])
                    pw_seen[j % 8] += 1
                    g = GSEQ[q0]
                    tab = 0 if g < GROUPS1 else 1
                    gl = g if tab == 0 else g - GROUPS1
                    if n == 2:
                        sync.dma_start(
                            out=pdst[tab][gl * 512:(gl + 2) * 512, :]
                                .rearrange("(g p j) d -> p g (j d)", g=2, p=128),
                            in_=pvall[:, (q0 % PVS) * 512:((q0 % PVS) + 2) * 512]
                                .bitcast(U32).rearrange("p (g x) -> p g x", g=2),
                        ).then_inc(s_pw[j % 8], 16)
                    else:
                        sync.dma_start(
                            out=pdst[tab][gl * 512:(gl + 1) * 512, :]
                                .rearrange("(p j) d -> p (j d)", p=128),
                            in_=pv[q0 % PVS].bitcast(U32),
                        ).then_inc(s_pw[j % 8], 16)
                if q == 30:
                    sync.dma_start(out=idxt[0][:, c4:idx_cols],
                                   in_=idxa[:, c4:idx_cols]).then_inc(s_idx2, 16)

            run_load_loop(sync, 0, body, loads_first=True)

            for bk in BORDER:
                sync.wait_ge(s_red, int(red_done[bk]))
                stot = bslot_tot[bk]
                sync.dma_start(out=res[bk][:, 0:stot],
                               in_=rt[bk][:, 0:stot]).then_inc(s_out, 16)
            sync.wait_ge(s_out, 16 * NBUCKET)

        # ------------------------------------------------ Act: embT half-1,
        # idxb, late PSUM -> bf16 drains
        @block.scalar
        def _(scalar):
            lc1 = make_load_chunk(scalar, 1)
            lc1(0)
            lc1(1)
            for k in range(2):
                scalar.dma_start(out=w2c[:, k * 128:(k + 1) * 128],
                                 in_=w2[k * 128:(k + 1) * 128, :]).then_inc(s_cw, 16)
            scalar.dma_start(out=btile[1][:], in_=b2t[:]).then_inc(s_cb, 16)
            scalar.dma_start(out=bt[1][:], in_=b2f[:]).then_inc(s_cl, 16)
            scalar.dma_start(out=onesr[:], in_=onesd[:]).then_inc(s_cl, 16)
            c4 = int(col_off_rank[min(4, NBUCKET)])
            scalar.dma_start(out=idxt[1][:, 0:c4],
                             in_=idxb[:, 0:c4]).then_inc(s_idx1, 16)

            da_next = [0]

            def body(q):
                if q == 30:
                    scalar.dma_start(out=idxt[1][:, c4:idx_cols],
                                     in_=idxb[:, c4:idx_cols]).then_inc(s_idx2, 16)
                while da_next[0] < len(DA_PLAN):
                    q0, n = DA_PLAN[da_next[0]]
                    if q0 + n - 1 > q:
                        break
                    da_next[0] += 1
                    scalar.wait_ge(s_bias, 4 * (q0 + n - DSPLIT))
                    for p in range(q0, q0 + n):
                        if p >= PVS:
                            wait_write(scalar, p - PVS)
                    scalar.activation(
                        out=pvall[:, (q0 % PVS) * 512:((q0 % PVS) + n) * 512],
                        in_=psv(q0, 0, n * 512),
                        func=mybir.ActivationFunctionType.Copy,
                    ).then_inc(s_da, 1)

            run_load_loop(scalar, 1, body)

        # ------------------------------------------------ PE: projections
        @block.tensor
        def _(tensor):
            tensor.wait_ge(s_cw, 4 * 16)
            for q, g in enumerate(GSEQ):
                if q == DSPLIT:
                    tensor.wait_ge(s_cl, 3 * 16)  # bt/onesr for bias matmuls
                tab = 0 if g < GROUPS1 else 1
                wc = w1c if tab == 0 else w2c
                if q >= 8:
                    wait_drained(tensor, q - 8)  # psum region free
                late_bias = q >= DSPLIT
                for j in range(4):
                    tg = g * 4 + j if tab == 0 else TILES1 + (g - GROUPS1) * 4 + j
                    cid, col0 = _chunk_of_tile(tg)
                    cq = CPOS[cid]
                    if col0 == 0:
                        tensor.wait_ge(s_load[cq % 4], 32 * (cq // 4 + 1))
                    out = psv(q, j * 128, (j + 1) * 128)
                    tensor.matmul(out=out, lhsT=et[cq % 4][0][:, col0:col0 + 128],
                                  rhs=wc[:, 0:128], start=True, stop=False).then_inc(s_mm, 1)
                    tensor.matmul(out=out, lhsT=et[cq % 4][1][:, col0:col0 + 128],
                                  rhs=wc[:, 128:256], start=False,
                                  stop=not late_bias).then_inc(s_mm, 1)
                    if late_bias:
                        tensor.matmul(out=out, lhsT=onesr[:],
                                      rhs=bt[tab][:, j * 128:(j + 1) * 128],
                                      start=False, stop=True).then_inc(s_bias, 1)

        # ------------------------------------------------ DVE: early drains,
        # then mul + tree levels 1-3 of the dot-product pipeline
        @block.vector
        def _(vector):
            vector.wait_ge(s_cb, 2 * 16)
            for q0, n in DD_PLAN:
                vector.wait_ge(s_mm, 8 * (q0 + n))
                for p in range(q0, q0 + n):
                    if p >= PVS:
                        wait_write(vector, p - PVS)
                tab = 0 if GSEQ[q0] < GROUPS1 else 1
                vector.tensor_add(
                    out=pvall[:, (q0 % PVS) * 512:((q0 % PVS) + n) * 512],
                    in0=psv(q0, 0, n * 512),
                    in1=btile[tab][:, 0:n * 512]).then_inc(s_dd, 1)

            def fold_view(k, w):
                gsz = flat[k][2]
                a = at[k % AT_BUFS][:, 0:gsz // 2].bitcast(BF16)
                return a.rearrange("p (s t d) -> p s t d", t=128 // w, d=w)

            def op_mul(k):
                bk, ci, gsz, g0 = flat[k]
                vector.wait_ge(s_ga[k % AT_BUFS], ga_tgt[k])
                vector.wait_ge(s_gb[k % BT_BUFS], gb_tgt[k])
                a = at[k % AT_BUFS][:, 0:gsz // 2].bitcast(BF16)
                b = btg[k % BT_BUFS][:, 0:gsz // 2].bitcast(BF16)
                vector.tensor_mul(out=a, in0=a, in1=b).then_inc(s_mul, 1)

            def op_l1(k):
                vector.wait_ge(s_mul, k + 1)
                v = fold_view(k, 64)
                vector.tensor_add(out=v[:, :, 0:1, :], in0=v[:, :, 0:1, :],
                                  in1=v[:, :, 1:2, :]).then_inc(s_t0, 1)

            def op_l2d(k):
                vector.wait_ge(s_t0, k + 1)
                v = fold_view(k, 32)
                vector.tensor_add(out=v[:, :, 0:1, :], in0=v[:, :, 0:1, :],
                                  in1=v[:, :, 1:2, :]).then_inc(s_t1d, 1)

            def op_red(k):
                bk, ci, gsz, g0 = flat[k]
                vector.wait_ge(s_t3, k + 1)
                w8 = fold_view(k, 8)
                s0 = g0 // 128
                vector.reduce_sum(
                    out=rt[bk][:, s0:s0 + gsz // 128]
                        .rearrange("p (s o) -> p s o", o=1),
                    in_=w8[:, :, 0:1, :],
                    axis=AX.X,
                ).then_inc(s_red, 1)

            with nc.allow_low_precision(reason="bf16 dot products; tol 2e-2"):
                for k in range(ncall + 6):
                    if k < ncall:
                        op_mul(k)
                    if 1 <= k < ncall + 1:
                        op_l1(k - 1)
                    if 2 <= k < ncall + 2:
                        if not pl2[k - 2]:
                            op_l2d(k - 2)
                    if 6 <= k:
                        op_red(k - 6)

        # ------------------------------------------------ Pool: gathers + L4
        # + reduce (library-switched)
        @block.gpsimd
        def _(gpsimd):
            from concourse import library_config
            greg = gpsimd.to_reg(GS)
            regs = {GS: greg}
            for bk in range(NBUCKET):
                for gsz in bcalls[bk]:
                    r = gsz % GS
                    if r and r not in regs:
                        regs[r] = gpsimd.to_reg(r)
            cur_lib = [None]

            def lib(l):
                if cur_lib[0] is not l:
                    gpsimd.load_library(l)
                    cur_lib[0] = l

            def gathers(k):
                bk, ci, gsz, g0 = flat[k]
                if ci == 0:
                    # bucket gate: idx present + both blocks' p-writes done
                    gpsimd.wait_ge(s_idx1 if CRANK[bk] < 4 else s_idx2, 32)
                    for r in range(8):
                        if BUCKET_PW_NEED[bk][r]:
                            gpsimd.wait_ge(s_pw[r], 16 * BUCKET_PW_NEED[bk][r])
                if k >= AT_BUFS:
                    gpsimd.wait_ge(s_red, k - AT_BUFS + 1)
                if k >= BT_BUFS:
                    gpsimd.wait_ge(s_mul, k - BT_BUFS + 1)
                fi, ti = bk // NTB, bk % NTB
                lib(library_config.mlp)
                for u in range(nsub[k]):
                    g = min(GS, gsz - u * GS)
                    c0 = col0_of[bk] + (g0 + u * GS) // 16
                    uo = u * GS * DU // 128
                    gpsimd.dma_gather(
                        out_ap=at[k % AT_BUFS][:, uo:uo + g * DU // 128]
                            .rearrange("p (s d) -> p s d", d=DU),
                        in_ap=p1d[fi * NBP1:(fi + 1) * NBP1, :],
                        idxs_ap=idxt[0][:, c0:c0 + g // 16],
                        num_idxs=g, num_idxs_reg=regs[g], elem_size=DU,
                        queue_num=0,
                    ).then_inc(s_ga[k % AT_BUFS], 16)
                    gpsimd.dma_gather(
                        out_ap=btg[k % BT_BUFS][:, uo:uo + g * DU // 128]
                            .rearrange("p (s d) -> p s d", d=DU),
                        in_ap=p2d[ti * NBP2:(ti + 1) * NBP2, :],
                        idxs_ap=idxt[1][:, c0:c0 + g // 16],
                        num_idxs=g, num_idxs_reg=regs[g], elem_size=DU,
                        queue_num=0,
                    ).then_inc(s_gb[k % BT_BUFS], 16)

            def pfold_view(k, w):
                gsz = flat[k][2]
                a = at[k % AT_BUFS][:, 0:gsz // 2].bitcast(BF16)
                return a.rearrange("p (s t d) -> p s t d", t=128 // w, d=w)

            def op_l2p(k):
                gpsimd.wait_ge(s_ga[k % AT_BUFS], ga_tgt[k])
                gpsimd.wait_ge(s_t0, k + 1)
                lib(library_config.standard)
                v = pfold_view(k, 32)
                gpsimd.tensor_add(out=v[:, :, 0:1, :], in0=v[:, :, 0:1, :],
                                  in1=v[:, :, 1:2, :]).then_inc(s_t1p, 1)

            def op_l3(k):
                gpsimd.wait_ge(s_ga[k % AT_BUFS], ga_tgt[k])
                gpsimd.wait_ge(s_t1p if pl2[k] else s_t1d, l2cnt[k])
                lib(library_config.standard)
                v = pfold_view(k, 16)
                gpsimd.tensor_add(out=v[:, :, 0:1, :], in0=v[:, :, 0:1, :],
                                  in1=v[:, :, 1:2, :]).then_inc(s_t2, 1)

            def op_l4(k):
                gpsimd.wait_ge(s_t2, k + 1)
                lib(library_config.standard)
                v = pfold_view(k, 8)
                gpsimd.tensor_add(out=v[:, :, 0:1, :], in0=v[:, :, 0:1, :],
                                  in1=v[:, :, 1:2, :]).then_inc(s_t3, 1)

            with nc.allow_low_precision(reason="bf16 dot products; tol 2e-2"):
                for k in range(ncall + 5):
                    if 3 <= k < ncall + 3 and pl2[k - 3]:
                        op_l2p(k - 3)
                    if 4 <= k < ncall + 4:
                        op_l3(k - 4)
                    if 5 <= k:
                        op_l4(k - 5)
                    if k < ncall:
                        gathers(k)

    return nc


_NC_CACHE = {}


def _get_nc(caps=None):
    global _NC_CACHE
    if caps is None:
        assert _NC_CACHE, "call _marshal first to determine caps"
        return next(iter(_NC_CACHE.values()))
    caps = tuple(caps)
    if caps not in _NC_CACHE:
        nc = build_bass(caps)
        from concourse.library_overlay import lower_extended_insts
        lower_extended_insts(nc)
        _NC_CACHE[caps] = nc
    return _NC_CACHE[caps]


# ---------------------------------------------------------------- host side
def _perm_rows(n):
    """local node id -> permuted table row (partition-major within 512)."""
    return (n // 512) * 512 + (n % 128) * 4 + (n % 512) // 128


def _marshal(emb_1, emb_2, nodes_from_to, W1, b1, W2, b2):
    """Shard/bucket inputs per core.  Returns (in_maps, books, caps)."""
    f = np.asarray(nodes_from_to[:, 0], dtype=np.int64)
    t = np.asarray(nodes_from_to[:, 1], dtype=np.int64)
    e1T = np.ascontiguousarray(
        np.asarray(emb_1, dtype=np.float32).T).astype(BFNP)
    e2T = np.ascontiguousarray(
        np.asarray(emb_2, dtype=np.float32).T).astype(BFNP)
    W1 = np.asarray(W1, dtype=np.float32).astype(BFNP)
    W2 = np.asarray(W2, dtype=np.float32).astype(BFNP)
    b1 = np.asarray(b1, dtype=np.float32).reshape(-1)
    b2 = np.asarray(b2, dtype=np.float32).reshape(-1)

    core = (f // (NFB * NB1)) * 4 + t // (NTB * NB2)
    order0 = np.argsort(core, kind="stable")
    ccnt = np.bincount(core, minlength=N_CORES)
    coff = np.concatenate([[0], np.cumsum(ccnt)])

    b1f = np.tile(b1.reshape(1, D_OUT), (1, 4)).astype(BFNP)
    b2f = np.tile(b2.reshape(1, D_OUT), (1, 4)).astype(BFNP)
    b1tt = np.tile(b1.reshape(1, D_OUT), (128, 8)).astype(BFNP)
    b2tt = np.tile(b2.reshape(1, D_OUT), (128, 8)).astype(BFNP)
    onesd = np.ones((1, 128), BFNP)

    # first pass: per-bucket counts across cores fix the shared program shape
    pre = []
    bmax = np.zeros(NBUCKET, np.int64)
    for c in range(N_CORES):
        a, b = c // 4, c % 4
        sel = order0[coff[c]:coff[c + 1]]
        fc, tcv = f[sel], t[sel]
        fi = fc // NB1 - NFB * a
        ti = tcv // NB2 - NTB * b
        fl = _perm_rows(fc % NB1).astype(np.int16)
        tl = _perm_rows(tcv % NB2).astype(np.int16)
        bk = fi * NTB + ti
        o2 = np.argsort(bk, kind="stable")
        cnts = np.bincount(bk, minlength=NBUCKET)
        bmax = np.maximum(bmax, cnts)
        pre.append((sel[o2], fl[o2], tl[o2], cnts))
    caps = tuple(int(-(-m // 128) * 128) for m in bmax)
    (bcalls, bslot_tot, col_off_rank, col0_of, slot_max, flat, red_done,
     nsub, ga_tgt, gb_tgt, pl2, l2cnt) = _layout(caps)
    idx_cols = int(col_off_rank[-1])

    in_maps, books = [], []
    for c in range(N_CORES):
        a, b = c // 4, c % 4
        sel2, fl2, tl2, cnts = pre[c]
        pos = np.concatenate([[0], np.cumsum(cnts)])

        idxa = np.zeros((128, idx_cols), np.int16)
        idxb = np.zeros((128, idx_cols), np.int16)
        for k in range(NBUCKET):
            cap_k = caps[k]
            sa = np.zeros(cap_k, np.int16)
            sbv = np.zeros(cap_k, np.int16)
            sa[:cnts[k]] = fl2[pos[k]:pos[k + 1]]
            sbv[:cnts[k]] = tl2[pos[k]:pos[k + 1]]
            # wrap by 16 within each sub-gather; replicated across 8 groups
            wa = np.zeros((16, cap_k // 16), np.int16)
            wb = np.zeros((16, cap_k // 16), np.int16)
            cc = 0
            while cc < cap_k:
                g = min(GS, cap_k - cc)
                seg = slice(cc, cc + g)
                wa[:, cc // 16:(cc + g) // 16] = sa[seg].reshape(g // 16, 16).T
                wb[:, cc // 16:(cc + g) // 16] = sbv[seg].reshape(g // 16, 16).T
                cc += g
            idxa[:, col0_of[k]:col0_of[k] + cap_k // 16] = np.tile(wa, (8, 1))
            idxb[:, col0_of[k]:col0_of[k] + cap_k // 16] = np.tile(wb, (8, 1))

        e1t = np.zeros((D_IN, P1_ROWS), BFNP)
        for i in range(NFB):
            blk = e1T[:, (NFB * a + i) * NB1:(NFB * a + i + 1) * NB1]
            e1t[:, i * NBP1:i * NBP1 + NB1] = blk
        e2t = np.zeros((D_IN, P2_ROWS), BFNP)
        for i in range(NTB):
            blk = e2T[:, (NTB * b + i) * NB2:(NTB * b + i + 1) * NB2]
            e2t[:, i * NBP2:i * NBP2 + NB2] = blk

        in_maps.append({
            "e1t": e1t, "e2t": e2t,
            "w1": W1, "w2": W2, "b1f": b1f, "b2f": b2f,
            "b1t": b1tt, "b2t": b2tt, "onesd": onesd,
            "idxa": idxa, "idxb": idxb,
        })
        books.append((sel2, cnts, pos))
    return in_maps, books, caps


def _unmarshal(results, books, caps, n_edges):
    out = np.empty(n_edges, np.float32)
    for c in range(N_CORES):
        sel2, cnts, pos = books[c]
        r = np.asarray(results[c]["res"]).astype(np.float32)
        for k in range(NBUCKET):
            if cnts[k] == 0:
                continue
            stream = r[k].T.reshape(-1)
            out[sel2[pos[k]:pos[k + 1]]] = stream[:cnts[k]]
    return out


def _run(inputs, trace=False, **run_kwargs):
    from concourse.bass_utils import run_bass_kernel_spmd

    in_maps, books, caps = _marshal(**inputs)
    nc = _get_nc(caps)
    r = run_bass_kernel_spmd(
        nc, in_maps, core_ids=list(range(N_CORES)), trace=trace, **run_kwargs
    )
    out = _unmarshal(r.results, books, caps, len(inputs["nodes_from_to"]))
    return out, r


def kernel(**inputs) -> np.ndarray:
    out, _ = _run(inputs, trace=False)
    return out
